# revision 17
# baseline (speedup 1.0000x reference)
"""Trainium2 Bass kernel for nn_Encoder_78795470012907.

Encoder layer: per-head Q/K/V projections, scores = QK^T/sqrt(dk),
double softmax (over batch axis, then over key axis), Z = pV, concat
heads, output projection. S=2048, B=4, D=512, H=8, dk=dv=64.

Sharding: head-parallel over 8 cores (core h owns head h) for the
attention; AllToAll re-shards by token for the output projection, so
each core emits a disjoint 1024-token slice of the output (host just
concatenates).

Layout notes (per core):
 - tokens are b-major: tok = b*2048 + s.
 - X is fed pre-transposed AND pre-cast to bf16 from host as
   XT [D, NTOK]; all projection weights are bf16 (fast LDW + 1 cyc/row
   matmuls vs ~480ns/mm for f32r).
 - projections produce Q^T/K^T [dk, tok] stacked in b-pairs so the
   scores matmuls row-pack two batches into the 128x128 PE array.
 - scores are computed transposed ([t, s] tiles); the softmax over the
   key axis t rides the Z matmul via a ones-column appended to V
   (row 64 of the Z psum accumulates sum_t exp).
 - softmax over b: e=exp(s/8) -> D=sum_b e -> r=1/D -> p1=e*r, with
   1/D on the custom-DVE fast reciprocal so ACT stays on one exp table
   set (a Reciprocal/Ln activation would thrash ACT_TABLE_LOAD per tile).
 - phase B is software-pipelined two blocks deep so exp1(g+2) precedes
   exp2(g) in the ACT queue; the AllToAll is split in two halves, the
   first overlapping the second half of the attention loop.
 - phase C: denominators ride the a2a as row 64 of each 65-row chunk;
   the reciprocal runs on an [8, 512] head-stacked tile (multi-partition
   DVE) and is broadcast to 64 rows with a K=8 selector matmul on the
   otherwise-idle PE, replacing the old Ln/Exp + PartitionBroadcast
   chain. Half-0's gather + reciprocal are interleaved into the tail of
   the attention loop so only half-1's chain sits after the last a2a.
"""

from contextlib import ExitStack

import numpy as np
import ml_dtypes

import concourse.bass as bass
import concourse.tile as tile
from concourse import bacc, mybir
from concourse.bass_utils import run_bass_kernel_spmd

S, B, D = 2048, 4, 512
H, DK, DV = 8, 64, 64
N_CORES = 8
NTOK = S * B          # 8192 tokens, b-major
TOKC = NTOK // N_CORES  # 1024 tokens per core for the output slice
SC = 512              # s-chunk (columns of a scores^T tile)
TC = 128              # t-chunk (partitions of a scores^T tile)
N_SC = S // SC        # 4
N_TC = S // TC        # 16
HT = 512              # tokens per a2a half (per core)

F32 = mybir.dt.float32
BF16 = mybir.dt.bfloat16
AF = mybir.ActivationFunctionType


def build_kernel():
    nc = bacc.Bacc(num_devices=N_CORES)

    # X pre-tiled on host: [128, chunk(16), dslice(4), 512] flattened so a
    # whole chunk is one contiguous-4KB-per-partition DMA.
    xt_d = nc.dram_tensor("xt", [128, 16 * 4 * 512], BF16, kind="ExternalInput")
    wqk_d = nc.dram_tensor("wqk", [D, 128], BF16, kind="ExternalInput")
    bqk_d = nc.dram_tensor("bqk", [128, 1], F32, kind="ExternalInput")
    wv_d = nc.dram_tensor("wv", [D, DV], BF16, kind="ExternalInput")
    wo_d = nc.dram_tensor("wo", [D, D], BF16, kind="ExternalInput")
    # bo' = concat_h(bV) @ WO + bO (V-bias folded through the out proj)
    bo_d = nc.dram_tensor("bo", [1, D], BF16, kind="ExternalInput")
    sel_d = nc.dram_tensor("sel", [8, 512], BF16, kind="ExternalInput")
    out_d = nc.dram_tensor("out", [TOKC, D], F32, kind="ExternalOutput")

    with tile.TileContext(nc) as tc, ExitStack() as ctx:
        pp = ctx.enter_context(tc.tile_pool(name="persist", bufs=1))
        dram = ctx.enter_context(tc.tile_pool(name="dram", bufs=1, space="DRAM"))

        # ---- persistent SBUF ----
        # Q^T/K^T in b-pairs: rows 0:64 = batch 2p, rows 64:128 = batch 2p+1
        qt = [pp.tile([128, S], BF16, tag=f"qt{p}", name=f"qt{p}") for p in range(2)]
        kt = [pp.tile([128, S], BF16, tag=f"kt{p}", name=f"kt{p}") for p in range(2)]
        # V-tilde: 64 token-chunks of [128 tok, 65] (col 64 = ones)
        vt = pp.tile([128, 64 * 65], BF16, tag="vt", name="vt")
        # Z^T (unnormalized) + denom row: [65, NTOK]
        zt = pp.tile([65, NTOK], BF16, tag="zt", name="zt")

        # weights (all bf16, DMA'd directly)
        wqk = [pp.tile([128, 128], BF16, tag=f"wqk{i}", name=f"wqk{i}") for i in range(4)]
        wv = [pp.tile([128, DV], BF16, tag=f"wv{i}", name=f"wv{i}") for i in range(4)]
        wo = [pp.tile([128, D], BF16, tag=f"wo{i}", name=f"wo{i}") for i in range(4)]
        bqk = pp.tile([128, 1], F32, tag="bqk", name="bqk")
        bo = pp.tile([1, D], BF16, tag="bo", name="bo")

        # chunk-0 critical-path DMAs first (wqk + bqk); bulk weights after
        for i in range(4):
            nc.sync.dma_start(wqk[i][:], wqk_d[i * 128:(i + 1) * 128, :])
        nc.sync.dma_start(bqk[:], bqk_d[:])

        def load_late_weights():
            for i in range(4):
                nc.sync.dma_start(wv[i][:], wv_d[i * 128:(i + 1) * 128, :])
            for i in range(4):
                nc.sync.dma_start(wo[i][:], wo_d[i * 128:(i + 1) * 128, :])
            nc.sync.dma_start(bo[:], bo_d[:])
            nc.sync.dma_start(sel[:], sel_d[:])

        # ones staging (memset bf16 directly is fine; f32 staging kept for
        # the f32-sensitive consumers)
        onesf = pp.tile([128, 128], F32, tag="onesf", name="onesf")
        nc.vector.memset(onesf[:], 1.0)
        ones_bf = pp.tile([1, 128], BF16, tag="ones_bf", name="ones_bf")
        nc.vector.tensor_copy(ones_bf[:], onesf[0:1, :])
        # ones column (col 64 of each 65-wide group) of V-tilde
        vt_ones = vt[:].rearrange("p (n c) -> p n c", c=65)[:, :, 64:65]
        nc.vector.tensor_copy(vt_ones, onesf[:, 0:64, None])
        # head-selector for the denominator broadcast: sel[p, j*64+k] =
        # (p == j); lhsT of a K=8 matmul that replicates row j of an
        # [8, 512] tile onto 64 output partitions. Host-provided constant.
        sel = pp.tile([8, 512], BF16, tag="sel", name="sel")

        # ================= Phase A: projections =================
        with (
            tc.tile_pool(name="xtp", bufs=2) as xp,
            tc.tile_pool(name="psA", bufs=2, space="PSUM") as psA,
        ):
            # b-inner order so the first 4 chunks cover (sc=0, t=0..3) of
            # every batch - lets attention start ~4x earlier
            for n_ck, ck in enumerate(
                    [b * 4 + ssub for ssub in range(4) for b in range(4)]):
                b = ck // 4
                pair, row = b // 2, (b % 2) * 64
                # one contiguous DMA for the whole chunk (4KB/partition)
                xbf = xp.tile([128, 4 * 512], BF16, tag="xb", name="xbf")
                nc.sync.dma_start(xbf[:], xt_d[:, ck * 2048:(ck + 1) * 2048])
                if n_ck == 0:
                    load_late_weights()
                # Q^T | K^T (stacked 64+64) for this token chunk
                pqk = psA.tile([128, 512], F32, tag="pqk", name="pqk")
                for i in range(4):
                    nc.tensor.matmul(pqk[:], wqk[i][:],
                                     xbf[:, i * 512:(i + 1) * 512],
                                     start=(i == 0), stop=(i == 3))
                scol = (ck % 4) * 512
                nc.scalar.activation(qt[pair][row:row + 64, scol:scol + 512],
                                     pqk[0:64, :], AF.Identity, bias=bqk[0:64, :])
                nc.scalar.activation(kt[pair][row:row + 64, scol:scol + 512],
                                     pqk[64:128, :], AF.Identity, bias=bqk[64:128, :])
                # V (natural layout) per 128-token subchunk; bV is folded
                # into the output-projection bias on the host, so no bias
                # matmul here.
                for sub in range(4):
                    pv = psA.tile([128, DV], F32, tag="pv", name="pv")
                    for i in range(4):
                        nc.tensor.matmul(
                            pv[:], xbf[:, i * 512 + sub * 128:i * 512 + (sub + 1) * 128],
                            wv[i][:], start=(i == 0), stop=(i == 3))
                    tci = ck * 4 + sub  # global token-chunk index (b-major)
                    nc.vector.tensor_copy(vt[:, tci * 65:tci * 65 + 64], pv[:])

        # ================= Phase B: attention (+ phase C half-0 prep) ====
        with tc.tile_pool(name="wc", bufs=2) as wcp:
            # phase-C tiles that must live from mid-attention into the tail
            zc = [wcp.tile([65, N_CORES * HT], BF16, tag=f"zc{q}", name=f"zc{q}")
                  for q in range(2)]
            rd8 = [wcp.tile([8, HT], BF16, tag=f"rd8{q}", name=f"rd8{q}")
                   for q in range(2)]
            rdf = [wcp.tile([8, HT], F32, tag=f"rdf{q}", name=f"rdf{q}")
                   for q in range(2)]
            rfp = [wcp.tile([8, HT], F32, tag=f"rfp{q}", name=f"rfp{q}")
                   for q in range(2)]
            r8b = [wcp.tile([8, HT], BF16, tag=f"r8b{q}", name=f"r8b{q}")
                   for q in range(2)]

            a2a_in_h = [dram.tile([N_CORES * 65, 512], BF16, tag=f"a2a_in{q}",
                                  name=f"a2a_in{q}") for q in range(2)]
            a2a_out_h = [dram.tile([N_CORES * 65, 512], BF16, tag=f"a2a_out{q}",
                                   name=f"a2a_out{q}") for q in range(2)]

            def emit_a2a(q):
                # chunk r = my head's Z^T cols for core r's half-q tokens:
                # tok = (r//2)*S + q*1024 + (r%2)*512 ... +512
                # Staged from the (idle) GpSimd queue: the collective's
                # completion-dependent descriptors then stay off the Sync
                # queue, whose FIFO carries the attention loop's semaphore
                # traffic (a Sync-queue block here stalls every engine).
                for r in range(N_CORES):
                    col = (r // 2) * S + q * 1024 + (r % 2) * 512
                    nc.gpsimd.dma_start(a2a_in_h[q][r * 65:(r + 1) * 65, :],
                                        zt[:, col:col + 512])
                nc.gpsimd.collective_compute(
                    "AllToAll",
                    mybir.AluOpType.bypass,
                    replica_groups=[list(range(N_CORES))],
                    ins=[a2a_in_h[q][:].opt()],
                    outs=[a2a_out_h[q][:].opt()],
                )

            def gather_half(q):
                """DMA half q out of a2a_out (waits on the collective)."""
                src = a2a_out_h[q][:].rearrange("(j p) s -> p j s", p=65)
                nc.sync.dma_start(rd8[q][:], src[64, :, :])
                nc.sync.dma_start(
                    zc[q][:].rearrange("p (j s) -> p j s", j=N_CORES),
                    src[0:65, :, :])

            def recip_half(q):
                """1/denom on DVE (no PSUM use)."""
                nc.vector.tensor_copy(rdf[q][:], rd8[q][:])
                nc.vector.reciprocal_approx_fast(rfp[q][:], rdf[q][:])
                nc.vector.tensor_copy(r8b[q][:], rfp[q][:])

            with (
                tc.tile_pool(name="wb", bufs=2) as wb,
                tc.tile_pool(name="psB", bufs=1, space="PSUM") as psB,
            ):
                # Software-pipelined over 64 global blocks g = sc*16 + t.
                # Per iteration: scores(g)+exp1(g) are emitted BEFORE the
                # DVE chain of g and exp2(g-1), so the ACT queue interleaves
                # exp1(g+1) ahead of exp2(g) and blocks overlap.
                NB = N_SC * N_TC
                pipe = {}  # g -> p1 tile

                def softmax_b(g):
                    """scores(g) -> e(g) -> p1(g) tiles (no exp2 yet)."""
                    sc, t = g // N_TC, g % N_TC
                    scp = psB.tile([128, 4 * SC], F32, tag="scp", name="scp")
                    for b in range(4):
                        pair, row = b // 2, (b % 2) * 64
                        nc.tensor.matmul(
                            scp[:, b * SC:(b + 1) * SC],
                            kt[pair][row:row + 64, t * TC:(t + 1) * TC],
                            qt[pair][row:row + 64, sc * SC:(sc + 1) * SC],
                            start=True, stop=True,
                        )
                    # e = exp(scores/8) for all 4 b
                    e = wb.tile([128, 4 * SC], BF16, tag="e", name="e", bufs=4)
                    nc.scalar.activation(e[:], scp[:], AF.Exp, scale=0.125)
                    # D = sum_b e ; r = 1/D (custom-DVE fast reciprocal keeps
                    # ACT on the single exp table set - no table thrashing)
                    t01 = wb.tile([128, 2 * SC], BF16, tag="t01", name="t01", bufs=2)
                    nc.vector.tensor_add(t01[:], e[:, 0:2 * SC], e[:, 2 * SC:4 * SC])
                    dd = wb.tile([128, SC], BF16, tag="dd", name="dd", bufs=2)
                    nc.vector.tensor_add(dd[:], t01[:, 0:SC], t01[:, SC:2 * SC])
                    ddf = wb.tile([128, SC], F32, tag="ddf", name="ddf", bufs=2)
                    nc.vector.tensor_copy(ddf[:], dd[:])
                    rf = wb.tile([128, SC], F32, tag="rf", name="rf", bufs=2)
                    nc.vector.reciprocal_approx_fast(rf[:], ddf[:])
                    rr = wb.tile([128, SC], BF16, tag="rr", name="rr", bufs=2)
                    nc.vector.tensor_copy(rr[:], rf[:])
                    # p1 = e * r, one TT with r broadcast along the 4-b free dim
                    p1 = wb.tile([128, 4 * SC], BF16, tag="p1", name="p1", bufs=4)
                    nc.vector.tensor_mul(
                        p1[:].rearrange("p (b s) -> p b s", b=4),
                        e[:].rearrange("p (b s) -> p b s", b=4),
                        rr[:, None, :].broadcast_to([128, 4, SC]),
                    )
                    pipe[g] = p1

                def exp2_and_z(g, zacc):
                    """exp2(g) + Z accumulation (ones-col -> sum_t in row 64)."""
                    t = g % N_TC
                    p1 = pipe.pop(g)
                    q = wb.tile([128, 4 * SC], BF16, tag="q", name="q", bufs=4)
                    nc.scalar.activation(q[:], p1[:], AF.Exp)
                    for b in range(4):
                        tci = b * 16 + t
                        nc.tensor.matmul(
                            zacc[:, b * SC:(b + 1) * SC],
                            vt[:, tci * 65:(tci + 1) * 65],
                            q[:, b * SC:(b + 1) * SC],
                            start=(t == 0), stop=(t == N_TC - 1),
                        )

                zaccs = {}
                for g in range(NB + 2):
                    if g < NB:
                        if g % N_TC == 0:
                            zaccs[g // N_TC] = psB.tile([65, 4 * SC], F32,
                                                        tag="zacc", name="zacc")
                        softmax_b(g)
                    if g >= 2:
                        gz = g - 2
                        za = zaccs[gz // N_TC]
                        exp2_and_z(gz, za)
                        if gz % N_TC == N_TC - 1:
                            # evacuate Z^T (+denominator row) to bf16
                            sc_done = gz // N_TC
                            for b in range(4):
                                col = b * S + sc_done * SC
                                nc.vector.tensor_copy(zt[:, col:col + SC],
                                                      za[:, b * SC:(b + 1) * SC])
                            if sc_done == 1:
                                emit_a2a(0)  # overlaps remaining attention
                            elif sc_done == 3:
                                emit_a2a(1)
                                # a2a_0 done long ago; these run as the
                                # last attention blocks drain
                                gather_half(0)
                                recip_half(0)

            # ============ Phase C: output projection (tail) ============
            with (
                tc.tile_pool(name="oc", bufs=2) as oc,
                tc.tile_pool(name="psC", bufs=2, space="PSUM") as psC,
            ):
                def tail_half(q):
                    # normalized Zc^T in hd-major pairs: tile i = heads 2i,2i+1;
                    # per-head 1/denom broadcast to 64 rows via selector matmul
                    zcn = [wcp.tile([128, HT], BF16, tag=f"zcn{i}",
                                    name=f"zcn{i}", bufs=2) for i in range(4)]
                    for j in range(N_CORES):
                        rbp = psC.tile([64, HT], F32, tag="rbp", name="rbp")
                        nc.tensor.matmul(rbp[:], sel[:, j * 64:(j + 1) * 64],
                                         r8b[q][:], start=True, stop=True)
                        nc.vector.tensor_mul(
                            zcn[j // 2][(j % 2) * 64:(j % 2) * 64 + 64, :],
                            zc[q][0:64, j * HT:(j + 1) * HT],
                            rbp[:],
                        )
                    for m in range(HT // 128):
                        po = psC.tile([128, D], F32, tag="po", name="po")
                        for i in range(4):
                            nc.tensor.matmul(po[:], zcn[i][:, m * 128:(m + 1) * 128],
                                             wo[i][:], start=(i == 0), stop=False)
                        nc.tensor.matmul(po[:], ones_bf[:], bo[:], start=False,
                                         stop=True)
                        ot = oc.tile([128, D], F32, tag="ot", name="ot")
                        nc.scalar.activation(ot[:], po[:], AF.Identity)
                        row = q * HT + m * 128
                        nc.sync.dma_start(out_d[row:row + 128, :], ot[:])

                # half-1's gather waits on a2a_1 inside the DMA queue while
                # half-0 computes; its DVE recip lands after half-0's muls.
                gather_half(1)
                tail_half(0)
                recip_half(1)
                tail_half(1)

    nc.compile()
    return nc


_NC_CACHE = None


def _get_nc():
    global _NC_CACHE
    if _NC_CACHE is None:
        _NC_CACHE = build_kernel()
    return _NC_CACHE


def kernel(X, WQ, bQ, WK, bK, WV, bV, WO, bO, _trace=False, _trace_kwargs=None):
    """Full inputs in, full output out. Shards internally across 8 cores."""
    BF = ml_dtypes.bfloat16
    X = np.asarray(X, dtype=np.float32)
    WQ, bQ = np.asarray(WQ, np.float32), np.asarray(bQ, np.float32)
    WK, bK = np.asarray(WK, np.float32), np.asarray(bK, np.float32)
    WV, bV = np.asarray(WV, np.float32), np.asarray(bV, np.float32)
    WO, bO = np.asarray(WO, np.float32), np.asarray(bO, np.float32)
    # [S,B,D] -> XT [D, NTOK] with b-major tokens (tok = b*S + s), bf16,
    # then pre-tiled [128, chunk(16), dslice(4), 512] so each chunk is one
    # contiguous DMA
    xt = X.transpose(2, 1, 0).reshape(D, NTOK).astype(BF)
    xt = np.ascontiguousarray(
        xt.reshape(4, 128, 16, 512).transpose(1, 2, 0, 3).reshape(128, 16 * 4 * 512))
    wo_b = np.ascontiguousarray(WO.astype(BF))
    # fold the V bias through the output projection:
    # out = (Z/denom) @ WO + (concat_h bV) @ WO + bO
    bo_eff = bV.reshape(H * DV) @ WO + bO
    bo_b = np.ascontiguousarray(bo_eff[None, :].astype(BF))
    sel_np = np.zeros((8, 512), dtype=np.float32)
    for j in range(8):
        sel_np[j, j * 64:(j + 1) * 64] = 1.0
    sel_b = np.ascontiguousarray(sel_np.astype(BF))
    in_maps = []
    for h in range(N_CORES):
        wqk = np.ascontiguousarray(
            np.concatenate([WQ[h], WK[h]], axis=1).astype(BF))
        bqk = np.ascontiguousarray(
            np.concatenate([bQ[h], bK[h]])[:, None], dtype=np.float32)
        in_maps.append({
            "xt": xt,
            "wqk": wqk,
            "bqk": bqk,
            "wv": np.ascontiguousarray(WV[h].astype(BF)),
            "wo": wo_b,
            "bo": bo_b,
            "sel": sel_b,
        })
    nc = _get_nc()
    res = run_bass_kernel_spmd(
        nc, in_maps, core_ids=list(range(N_CORES)),
        trace=_trace, **(_trace_kwargs or {}),
    )
    # core c rows: [0:512] = tokens (c//2)*S + (c%2)*512 .. ; [512:1024] same + 1024
    fullb = np.empty((B, S, D), dtype=np.float32)
    for c in range(N_CORES):
        oc = res.results[c]["out"]
        b, off = c // 2, (c % 2) * 512
        fullb[b, off:off + 512] = oc[0:512]
        fullb[b, 1024 + off:1024 + off + 512] = oc[512:1024]
    full = fullb.transpose(1, 0, 2)
    if _trace:
        return np.ascontiguousarray(full), res
    return np.ascontiguousarray(full)


# revision 18
# speedup vs baseline: 1.4414x; 1.4414x over previous
"""Trainium2 Bass kernel for nn_Encoder_78795470012907.

Encoder layer: per-head Q/K/V projections, scores = QK^T/sqrt(dk),
double softmax (over batch axis, then over key axis), Z = pV, concat
heads, output projection. S=2048, B=4, D=512, H=8, dk=dv=64.

Sharding: head-parallel over 8 cores (core h owns head h) for the
attention; AllToAll re-shards by token for the output projection, so
each core emits a disjoint 1024-token slice of the output (host just
concatenates).

Layout notes (per core):
 - tokens are b-major: tok = b*2048 + s.
 - X is fed pre-transposed AND pre-cast to bf16 from host as
   XT [D, NTOK]; all projection weights are bf16 (fast LDW + 1 cyc/row
   matmuls vs ~480ns/mm for f32r).
 - projections produce Q^T/K^T [dk, tok] stacked in b-pairs so the
   scores matmuls row-pack two batches into the 128x128 PE array.
 - scores are computed transposed ([t, s] tiles); the softmax over the
   key axis t rides the Z matmul via a ones-column appended to V
   (row 64 of the Z psum accumulates sum_t exp).
 - softmax over b: e=exp(s/8) -> D=sum_b e -> r=1/D -> p1=e*r, with
   1/D on the custom-DVE fast reciprocal so ACT stays on one exp table
   set (a Reciprocal/Ln activation would thrash ACT_TABLE_LOAD per tile).
 - phase B is software-pipelined two blocks deep so exp1(g+2) precedes
   exp2(g) in the ACT queue; the AllToAll is split in two halves, the
   first overlapping the second half of the attention loop.
 - phase C: denominators ride the a2a as row 64 of each 65-row chunk;
   the reciprocal runs on an [8, 512] head-stacked tile (multi-partition
   DVE) and is broadcast to 64 rows with a K=8 selector matmul on the
   otherwise-idle PE, replacing the old Ln/Exp + PartitionBroadcast
   chain. Half-0's gather + reciprocal are interleaved into the tail of
   the attention loop so only half-1's chain sits after the last a2a.
"""

from contextlib import ExitStack

import numpy as np
import ml_dtypes

import concourse.bass as bass
import concourse.tile as tile
from concourse import bacc, mybir
from concourse.bass_utils import run_bass_kernel_spmd

S, B, D = 2048, 4, 512
H, DK, DV = 8, 64, 64
N_CORES = 8
NTOK = S * B          # 8192 tokens, b-major
TOKC = NTOK // N_CORES  # 1024 tokens per core for the output slice
SC = 512              # s-chunk (columns of a scores^T tile)
TC = 128              # t-chunk (partitions of a scores^T tile)
N_SC = S // SC        # 4
N_TC = S // TC        # 16
HT = 512              # tokens per a2a half (per core)

F32 = mybir.dt.float32
BF16 = mybir.dt.bfloat16
AF = mybir.ActivationFunctionType


def build_kernel():
    nc = bacc.Bacc(num_devices=N_CORES)

    # X pre-tiled on host: [128, chunk(16), dslice(4), 512] flattened so a
    # whole chunk is one contiguous-4KB-per-partition DMA.
    xt_d = nc.dram_tensor("xt", [128, 16 * 4 * 512], BF16, kind="ExternalInput")
    wqk_d = nc.dram_tensor("wqk", [D, 128], BF16, kind="ExternalInput")
    bqk_d = nc.dram_tensor("bqk", [128, 1], F32, kind="ExternalInput")
    wv_d = nc.dram_tensor("wv", [D, DV], BF16, kind="ExternalInput")
    wo_d = nc.dram_tensor("wo", [D, D], BF16, kind="ExternalInput")
    # bo' = concat_h(bV) @ WO + bO (V-bias folded through the out proj)
    bo_d = nc.dram_tensor("bo", [1, D], BF16, kind="ExternalInput")
    sel_d = nc.dram_tensor("sel", [8, 512], BF16, kind="ExternalInput")
    out_d = nc.dram_tensor("out", [TOKC, D], F32, kind="ExternalOutput")

    with tile.TileContext(nc) as tc, ExitStack() as ctx:
        pp = ctx.enter_context(tc.tile_pool(name="persist", bufs=1))
        dram = ctx.enter_context(tc.tile_pool(name="dram", bufs=1, space="DRAM"))

        # ---- persistent SBUF ----
        # Q^T/K^T in b-pairs: rows 0:64 = batch 2p, rows 64:128 = batch 2p+1
        qt = [pp.tile([128, S], BF16, tag=f"qt{p}", name=f"qt{p}") for p in range(2)]
        kt = [pp.tile([128, S], BF16, tag=f"kt{p}", name=f"kt{p}") for p in range(2)]
        # V-tilde: 64 token-chunks of [128 tok, 65] (col 64 = ones)
        vt = pp.tile([128, 64 * 65], BF16, tag="vt", name="vt")
        # Z^T (unnormalized) + denom row: [65, NTOK]
        zt = pp.tile([65, NTOK], BF16, tag="zt", name="zt")

        # weights (all bf16, DMA'd directly)
        wqk = [pp.tile([128, 128], BF16, tag=f"wqk{i}", name=f"wqk{i}") for i in range(4)]
        wv = [pp.tile([128, DV], BF16, tag=f"wv{i}", name=f"wv{i}") for i in range(4)]
        wo = [pp.tile([128, D], BF16, tag=f"wo{i}", name=f"wo{i}") for i in range(4)]
        bqk = pp.tile([128, 1], F32, tag="bqk", name="bqk")
        bo = pp.tile([1, D], BF16, tag="bo", name="bo")

        # chunk-0 critical-path DMAs first (wqk + bqk); bulk weights after
        for i in range(4):
            nc.sync.dma_start(wqk[i][:], wqk_d[i * 128:(i + 1) * 128, :])
        nc.sync.dma_start(bqk[:], bqk_d[:])

        def load_late_weights():
            for i in range(4):
                nc.sync.dma_start(wv[i][:], wv_d[i * 128:(i + 1) * 128, :])
            for i in range(4):
                nc.sync.dma_start(wo[i][:], wo_d[i * 128:(i + 1) * 128, :])
            nc.sync.dma_start(bo[:], bo_d[:])
            nc.sync.dma_start(sel[:], sel_d[:])

        # ones staging (memset bf16 directly is fine; f32 staging kept for
        # the f32-sensitive consumers)
        onesf = pp.tile([128, 128], F32, tag="onesf", name="onesf")
        nc.vector.memset(onesf[:], 1.0)
        ones_bf = pp.tile([1, 128], BF16, tag="ones_bf", name="ones_bf")
        nc.vector.tensor_copy(ones_bf[:], onesf[0:1, :])
        # ones column (col 64 of each 65-wide group) of V-tilde
        vt_ones = vt[:].rearrange("p (n c) -> p n c", c=65)[:, :, 64:65]
        nc.vector.tensor_copy(vt_ones, onesf[:, 0:64, None])
        # head-selector for the denominator broadcast: sel[p, j*64+k] =
        # (p == j); lhsT of a K=8 matmul that replicates row j of an
        # [8, 512] tile onto 64 output partitions. Host-provided constant.
        sel = pp.tile([8, 512], BF16, tag="sel", name="sel")

        # ================= Phase A: projections =================
        with (
            tc.tile_pool(name="xtp", bufs=2) as xp,
            tc.tile_pool(name="psA", bufs=2, space="PSUM") as psA,
        ):
            # b-inner order so the first 4 chunks cover (sc=0, t=0..3) of
            # every batch - lets attention start ~4x earlier
            for n_ck, ck in enumerate(
                    [b * 4 + ssub for ssub in range(4) for b in range(4)]):
                b = ck // 4
                pair, row = b // 2, (b % 2) * 64
                # one contiguous DMA for the whole chunk (4KB/partition)
                xbf = xp.tile([128, 4 * 512], BF16, tag="xb", name="xbf")
                nc.sync.dma_start(xbf[:], xt_d[:, ck * 2048:(ck + 1) * 2048])
                if n_ck == 0:
                    load_late_weights()
                # Q^T | K^T (stacked 64+64) for this token chunk
                pqk = psA.tile([128, 512], F32, tag="pqk", name="pqk")
                for i in range(4):
                    nc.tensor.matmul(pqk[:], wqk[i][:],
                                     xbf[:, i * 512:(i + 1) * 512],
                                     start=(i == 0), stop=(i == 3))
                scol = (ck % 4) * 512
                nc.scalar.activation(qt[pair][row:row + 64, scol:scol + 512],
                                     pqk[0:64, :], AF.Identity, bias=bqk[0:64, :])
                nc.scalar.activation(kt[pair][row:row + 64, scol:scol + 512],
                                     pqk[64:128, :], AF.Identity, bias=bqk[64:128, :])
                # V (natural layout) per 128-token subchunk; bV is folded
                # into the output-projection bias on the host, so no bias
                # matmul here.
                for sub in range(4):
                    pv = psA.tile([128, DV], F32, tag="pv", name="pv")
                    for i in range(4):
                        nc.tensor.matmul(
                            pv[:], xbf[:, i * 512 + sub * 128:i * 512 + (sub + 1) * 128],
                            wv[i][:], start=(i == 0), stop=(i == 3))
                    tci = ck * 4 + sub  # global token-chunk index (b-major)
                    nc.vector.tensor_copy(vt[:, tci * 65:tci * 65 + 64], pv[:])

        # ================= Phase B: attention (+ phase C half-0 prep) ====
        with tc.tile_pool(name="wc", bufs=2) as wcp:
            # phase-C tiles that must live from mid-attention into the tail
            zc = [wcp.tile([65, N_CORES * HT], BF16, tag=f"zc{q}", name=f"zc{q}")
                  for q in range(2)]
            rd8 = [wcp.tile([8, HT], BF16, tag=f"rd8{q}", name=f"rd8{q}")
                   for q in range(2)]
            rdf = [wcp.tile([8, HT], F32, tag=f"rdf{q}", name=f"rdf{q}")
                   for q in range(2)]
            rfp = [wcp.tile([8, HT], F32, tag=f"rfp{q}", name=f"rfp{q}")
                   for q in range(2)]
            r8b = [wcp.tile([8, HT], BF16, tag=f"r8b{q}", name=f"r8b{q}")
                   for q in range(2)]

            a2a_in_h = [dram.tile([N_CORES * 65, 512], BF16, tag=f"a2a_in{q}",
                                  name=f"a2a_in{q}") for q in range(2)]
            a2a_out_h = [dram.tile([N_CORES * 65, 512], BF16, tag=f"a2a_out{q}",
                                   name=f"a2a_out{q}") for q in range(2)]

            def emit_a2a(q):
                # chunk r = my head's Z^T cols for core r's half-q tokens:
                # tok = (r//2)*S + q*1024 + (r%2)*512 ... +512
                for r in range(N_CORES):
                    col = (r // 2) * S + q * 1024 + (r % 2) * 512
                    nc.sync.dma_start(a2a_in_h[q][r * 65:(r + 1) * 65, :],
                                        zt[:, col:col + 512])
                nc.gpsimd.collective_compute(
                    "AllToAll",
                    mybir.AluOpType.bypass,
                    replica_groups=[list(range(N_CORES))],
                    ins=[a2a_in_h[q][:].opt()],
                    outs=[a2a_out_h[q][:].opt()],
                )

            def gather_half(q):
                """DMA half q out of a2a_out (waits on the collective)."""
                src = a2a_out_h[q][:].rearrange("(j p) s -> p j s", p=65)
                nc.sync.dma_start(rd8[q][:], src[64, :, :])
                nc.sync.dma_start(
                    zc[q][:].rearrange("p (j s) -> p j s", j=N_CORES),
                    src[0:65, :, :])

            def recip_half(q):
                """1/denom on DVE (no PSUM use)."""
                nc.vector.tensor_copy(rdf[q][:], rd8[q][:])
                nc.vector.reciprocal_approx_fast(rfp[q][:], rdf[q][:])
                nc.vector.tensor_copy(r8b[q][:], rfp[q][:])

            with (
                tc.tile_pool(name="wb", bufs=2) as wb,
                tc.tile_pool(name="psB", bufs=1, space="PSUM") as psB,
            ):
                # Software-pipelined over 64 global blocks g = sc*16 + t.
                # Per iteration: scores(g)+exp1(g) are emitted BEFORE the
                # DVE chain of g and exp2(g-1), so the ACT queue interleaves
                # exp1(g+1) ahead of exp2(g) and blocks overlap.
                NB = N_SC * N_TC
                pipe = {}  # g -> p1 tile

                def softmax_b(g):
                    """scores(g) -> e(g) -> p1(g) tiles (no exp2 yet)."""
                    sc, t = g // N_TC, g % N_TC
                    scp = psB.tile([128, 4 * SC], F32, tag="scp", name="scp")
                    for b in range(4):
                        pair, row = b // 2, (b % 2) * 64
                        nc.tensor.matmul(
                            scp[:, b * SC:(b + 1) * SC],
                            kt[pair][row:row + 64, t * TC:(t + 1) * TC],
                            qt[pair][row:row + 64, sc * SC:(sc + 1) * SC],
                            start=True, stop=True,
                        )
                    # e = exp(scores/8) for all 4 b
                    e = wb.tile([128, 4 * SC], BF16, tag="e", name="e", bufs=4)
                    nc.scalar.activation(e[:], scp[:], AF.Exp, scale=0.125)
                    # D = sum_b e ; r = 1/D (custom-DVE fast reciprocal keeps
                    # ACT on the single exp table set - no table thrashing)
                    t01 = wb.tile([128, 2 * SC], BF16, tag="t01", name="t01", bufs=2)
                    nc.vector.tensor_add(t01[:], e[:, 0:2 * SC], e[:, 2 * SC:4 * SC])
                    dd = wb.tile([128, SC], BF16, tag="dd", name="dd", bufs=2)
                    nc.vector.tensor_add(dd[:], t01[:, 0:SC], t01[:, SC:2 * SC])
                    ddf = wb.tile([128, SC], F32, tag="ddf", name="ddf", bufs=2)
                    nc.vector.tensor_copy(ddf[:], dd[:])
                    rf = wb.tile([128, SC], F32, tag="rf", name="rf", bufs=2)
                    nc.vector.reciprocal_approx_fast(rf[:], ddf[:])
                    rr = wb.tile([128, SC], BF16, tag="rr", name="rr", bufs=2)
                    nc.vector.tensor_copy(rr[:], rf[:])
                    # p1 = e * r, one TT with r broadcast along the 4-b free dim
                    p1 = wb.tile([128, 4 * SC], BF16, tag="p1", name="p1", bufs=4)
                    nc.vector.tensor_mul(
                        p1[:].rearrange("p (b s) -> p b s", b=4),
                        e[:].rearrange("p (b s) -> p b s", b=4),
                        rr[:, None, :].broadcast_to([128, 4, SC]),
                    )
                    pipe[g] = p1

                def exp2_and_z(g, zacc):
                    """exp2(g) + Z accumulation (ones-col -> sum_t in row 64)."""
                    t = g % N_TC
                    p1 = pipe.pop(g)
                    q = wb.tile([128, 4 * SC], BF16, tag="q", name="q", bufs=4)
                    nc.scalar.activation(q[:], p1[:], AF.Exp)
                    for b in range(4):
                        tci = b * 16 + t
                        nc.tensor.matmul(
                            zacc[:, b * SC:(b + 1) * SC],
                            vt[:, tci * 65:(tci + 1) * 65],
                            q[:, b * SC:(b + 1) * SC],
                            start=(t == 0), stop=(t == N_TC - 1),
                        )

                zaccs = {}
                for g in range(NB + 2):
                    if g < NB:
                        if g % N_TC == 0:
                            zaccs[g // N_TC] = psB.tile([65, 4 * SC], F32,
                                                        tag="zacc", name="zacc")
                        softmax_b(g)
                    if g >= 2:
                        gz = g - 2
                        za = zaccs[gz // N_TC]
                        exp2_and_z(gz, za)
                        if gz % N_TC == N_TC - 1:
                            # evacuate Z^T (+denominator row) to bf16
                            sc_done = gz // N_TC
                            for b in range(4):
                                col = b * S + sc_done * SC
                                nc.vector.tensor_copy(zt[:, col:col + SC],
                                                      za[:, b * SC:(b + 1) * SC])
                            if sc_done == 1:
                                emit_a2a(0)  # overlaps remaining attention
                            elif sc_done == 3:
                                emit_a2a(1)
                                # a2a_0 done long ago; these run as the
                                # last attention blocks drain
                                gather_half(0)
                                recip_half(0)

            # ============ Phase C: output projection (tail) ============
            with (
                tc.tile_pool(name="oc", bufs=2) as oc,
                tc.tile_pool(name="psC", bufs=2, space="PSUM") as psC,
            ):
                def tail_half(q):
                    # normalized Zc^T in hd-major pairs: tile i = heads 2i,2i+1;
                    # per-head 1/denom broadcast to 64 rows via selector matmul
                    zcn = [wcp.tile([128, HT], BF16, tag=f"zcn{i}",
                                    name=f"zcn{i}", bufs=2) for i in range(4)]
                    for j in range(N_CORES):
                        rbp = psC.tile([64, HT], F32, tag="rbp", name="rbp")
                        nc.tensor.matmul(rbp[:], sel[:, j * 64:(j + 1) * 64],
                                         r8b[q][:], start=True, stop=True)
                        nc.vector.tensor_mul(
                            zcn[j // 2][(j % 2) * 64:(j % 2) * 64 + 64, :],
                            zc[q][0:64, j * HT:(j + 1) * HT],
                            rbp[:],
                        )
                    for m in range(HT // 128):
                        po = psC.tile([128, D], F32, tag="po", name="po")
                        for i in range(4):
                            nc.tensor.matmul(po[:], zcn[i][:, m * 128:(m + 1) * 128],
                                             wo[i][:], start=(i == 0), stop=False)
                        nc.tensor.matmul(po[:], ones_bf[:], bo[:], start=False,
                                         stop=True)
                        ot = oc.tile([128, D], F32, tag="ot", name="ot")
                        nc.scalar.activation(ot[:], po[:], AF.Identity)
                        row = q * HT + m * 128
                        nc.sync.dma_start(out_d[row:row + 128, :], ot[:])

                # half-1's gather waits on a2a_1 inside the DMA queue while
                # half-0 computes; its DVE recip lands after half-0's muls.
                gather_half(1)
                tail_half(0)
                recip_half(1)
                tail_half(1)

    nc.compile()
    return nc


_NC_CACHE = None


def _get_nc():
    global _NC_CACHE
    if _NC_CACHE is None:
        _NC_CACHE = build_kernel()
    return _NC_CACHE


def kernel(X, WQ, bQ, WK, bK, WV, bV, WO, bO, _trace=False, _trace_kwargs=None):
    """Full inputs in, full output out. Shards internally across 8 cores."""
    BF = ml_dtypes.bfloat16
    X = np.asarray(X, dtype=np.float32)
    WQ, bQ = np.asarray(WQ, np.float32), np.asarray(bQ, np.float32)
    WK, bK = np.asarray(WK, np.float32), np.asarray(bK, np.float32)
    WV, bV = np.asarray(WV, np.float32), np.asarray(bV, np.float32)
    WO, bO = np.asarray(WO, np.float32), np.asarray(bO, np.float32)
    # [S,B,D] -> XT [D, NTOK] with b-major tokens (tok = b*S + s), bf16,
    # then pre-tiled [128, chunk(16), dslice(4), 512] so each chunk is one
    # contiguous DMA
    xt = X.transpose(2, 1, 0).reshape(D, NTOK).astype(BF)
    xt = np.ascontiguousarray(
        xt.reshape(4, 128, 16, 512).transpose(1, 2, 0, 3).reshape(128, 16 * 4 * 512))
    wo_b = np.ascontiguousarray(WO.astype(BF))
    # fold the V bias through the output projection:
    # out = (Z/denom) @ WO + (concat_h bV) @ WO + bO
    bo_eff = bV.reshape(H * DV) @ WO + bO
    bo_b = np.ascontiguousarray(bo_eff[None, :].astype(BF))
    sel_np = np.zeros((8, 512), dtype=np.float32)
    for j in range(8):
        sel_np[j, j * 64:(j + 1) * 64] = 1.0
    sel_b = np.ascontiguousarray(sel_np.astype(BF))
    in_maps = []
    for h in range(N_CORES):
        wqk = np.ascontiguousarray(
            np.concatenate([WQ[h], WK[h]], axis=1).astype(BF))
        bqk = np.ascontiguousarray(
            np.concatenate([bQ[h], bK[h]])[:, None], dtype=np.float32)
        in_maps.append({
            "xt": xt,
            "wqk": wqk,
            "bqk": bqk,
            "wv": np.ascontiguousarray(WV[h].astype(BF)),
            "wo": wo_b,
            "bo": bo_b,
            "sel": sel_b,
        })
    nc = _get_nc()
    res = run_bass_kernel_spmd(
        nc, in_maps, core_ids=list(range(N_CORES)),
        trace=_trace, **(_trace_kwargs or {}),
    )
    # core c rows: [0:512] = tokens (c//2)*S + (c%2)*512 .. ; [512:1024] same + 1024
    fullb = np.empty((B, S, D), dtype=np.float32)
    for c in range(N_CORES):
        oc = res.results[c]["out"]
        b, off = c // 2, (c % 2) * 512
        fullb[b, off:off + 512] = oc[0:512]
        fullb[b, 1024 + off:1024 + off + 512] = oc[512:1024]
    full = fullb.transpose(1, 0, 2)
    if _trace:
        return np.ascontiguousarray(full), res
    return np.ascontiguousarray(full)


# revision 19
# speedup vs baseline: 1.4655x; 1.0167x over previous
"""Trainium2 Bass kernel for nn_Encoder_78795470012907.

Encoder layer: per-head Q/K/V projections, scores = QK^T/sqrt(dk),
double softmax (over batch axis, then over key axis), Z = pV, concat
heads, output projection. S=2048, B=4, D=512, H=8, dk=dv=64.

Sharding: head-parallel over 8 cores (core h owns head h) for the
attention; AllToAll re-shards by token for the output projection, so
each core emits a disjoint 1024-token slice of the output (host just
concatenates).

Layout notes (per core):
 - tokens are b-major: tok = b*2048 + s.
 - X is fed pre-transposed AND pre-cast to bf16 from host as
   XT [D, NTOK]; all projection weights are bf16 (fast LDW + 1 cyc/row
   matmuls vs ~480ns/mm for f32r).
 - projections produce Q^T/K^T [dk, tok] stacked in b-pairs so the
   scores matmuls row-pack two batches into the 128x128 PE array.
 - scores are computed transposed ([t, s] tiles); the softmax over the
   key axis t rides the Z matmul via a ones-column appended to V
   (row 64 of the Z psum accumulates sum_t exp).
 - softmax over b: e=exp(s/8) -> D=sum_b e -> r=1/D -> p1=e*r, with
   1/D on the custom-DVE fast reciprocal so ACT stays on one exp table
   set (a Reciprocal/Ln activation would thrash ACT_TABLE_LOAD per tile).
 - phase B is software-pipelined two blocks deep so exp1(g+2) precedes
   exp2(g) in the ACT queue; the AllToAll is split in two halves, the
   first overlapping the second half of the attention loop.
 - phase C: denominators ride the a2a as row 64 of each 65-row chunk;
   the reciprocal runs on an [8, 512] head-stacked tile (multi-partition
   DVE) and is broadcast to 64 rows with a K=8 selector matmul on the
   otherwise-idle PE, replacing the old Ln/Exp + PartitionBroadcast
   chain. Half-0's gather + reciprocal are interleaved into the tail of
   the attention loop so only half-1's chain sits after the last a2a.
"""

from contextlib import ExitStack

import numpy as np
import ml_dtypes

import concourse.bass as bass
import concourse.tile as tile
from concourse import bacc, mybir
from concourse.bass_utils import run_bass_kernel_spmd

S, B, D = 2048, 4, 512
H, DK, DV = 8, 64, 64
N_CORES = 8
NTOK = S * B          # 8192 tokens, b-major
TOKC = NTOK // N_CORES  # 1024 tokens per core for the output slice
SC = 512              # s-chunk (columns of a scores^T tile)
TC = 128              # t-chunk (partitions of a scores^T tile)
N_SC = S // SC        # 4
N_TC = S // TC        # 16
HT = 512              # tokens per a2a half (per core)

F32 = mybir.dt.float32
BF16 = mybir.dt.bfloat16
AF = mybir.ActivationFunctionType


def build_kernel():
    nc = bacc.Bacc(num_devices=N_CORES)

    # X pre-tiled on host: [128, chunk(16), dslice(4), 512] flattened so a
    # whole chunk is one contiguous-4KB-per-partition DMA.
    xt_d = nc.dram_tensor("xt", [128, 16 * 4 * 512], BF16, kind="ExternalInput")
    wqk_d = nc.dram_tensor("wqk", [D, 128], BF16, kind="ExternalInput")
    bqk_d = nc.dram_tensor("bqk", [128, 1], F32, kind="ExternalInput")
    wv_d = nc.dram_tensor("wv", [D, DV], BF16, kind="ExternalInput")
    wo_d = nc.dram_tensor("wo", [D, D], BF16, kind="ExternalInput")
    # bo' = concat_h(bV) @ WO + bO (V-bias folded through the out proj)
    bo_d = nc.dram_tensor("bo", [1, D], BF16, kind="ExternalInput")
    sel_d = nc.dram_tensor("sel", [8, 512], BF16, kind="ExternalInput")
    out_d = nc.dram_tensor("out", [TOKC, D], F32, kind="ExternalOutput")

    with tile.TileContext(nc) as tc, ExitStack() as ctx:
        pp = ctx.enter_context(tc.tile_pool(name="persist", bufs=1))
        dram = ctx.enter_context(tc.tile_pool(name="dram", bufs=1, space="DRAM"))

        # ---- persistent SBUF ----
        # Q^T/K^T in b-pairs: rows 0:64 = batch 2p, rows 64:128 = batch 2p+1
        qt = [pp.tile([128, S], BF16, tag=f"qt{p}", name=f"qt{p}") for p in range(2)]
        kt = [pp.tile([128, S], BF16, tag=f"kt{p}", name=f"kt{p}") for p in range(2)]
        # V-tilde: 64 token-chunks of [128 tok, 65] (col 64 = ones)
        vt = pp.tile([128, 64 * 65], BF16, tag="vt", name="vt")
        # Z^T (unnormalized) + denom row: [65, NTOK]
        zt = pp.tile([65, NTOK], BF16, tag="zt", name="zt")

        # weights (all bf16, DMA'd directly)
        wqk = [pp.tile([128, 128], BF16, tag=f"wqk{i}", name=f"wqk{i}") for i in range(4)]
        wv = [pp.tile([128, DV], BF16, tag=f"wv{i}", name=f"wv{i}") for i in range(4)]
        wo = [pp.tile([128, D], BF16, tag=f"wo{i}", name=f"wo{i}") for i in range(4)]
        bqk = pp.tile([128, 1], F32, tag="bqk", name="bqk")
        bo = pp.tile([1, D], BF16, tag="bo", name="bo")

        # chunk-0 critical-path DMAs first (wqk + bqk); bulk weights after
        for i in range(4):
            nc.sync.dma_start(wqk[i][:], wqk_d[i * 128:(i + 1) * 128, :])
        nc.sync.dma_start(bqk[:], bqk_d[:])

        def load_late_weights():
            for i in range(4):
                nc.sync.dma_start(wv[i][:], wv_d[i * 128:(i + 1) * 128, :])
            for i in range(4):
                nc.sync.dma_start(wo[i][:], wo_d[i * 128:(i + 1) * 128, :])
            nc.sync.dma_start(bo[:], bo_d[:])
            nc.sync.dma_start(sel[:], sel_d[:])

        # ones staging (memset bf16 directly is fine; f32 staging kept for
        # the f32-sensitive consumers)
        onesf = pp.tile([128, 128], F32, tag="onesf", name="onesf")
        nc.vector.memset(onesf[:], 1.0)
        ones_bf = pp.tile([1, 128], BF16, tag="ones_bf", name="ones_bf")
        nc.vector.tensor_copy(ones_bf[:], onesf[0:1, :])
        # ones column (col 64 of each 65-wide group) of V-tilde
        vt_ones = vt[:].rearrange("p (n c) -> p n c", c=65)[:, :, 64:65]
        nc.vector.tensor_copy(vt_ones, onesf[:, 0:64, None])
        # head-selector for the denominator broadcast: sel[p, j*64+k] =
        # (p == j); lhsT of a K=8 matmul that replicates row j of an
        # [8, 512] tile onto 64 output partitions. Host-provided constant.
        sel = pp.tile([8, 512], BF16, tag="sel", name="sel")

        # ================= Phase A: projections =================
        with (
            tc.tile_pool(name="xtp", bufs=2) as xp,
            tc.tile_pool(name="psA", bufs=2, space="PSUM") as psA,
        ):
            # b-inner order so the first 4 chunks cover (sc=0, t=0..3) of
            # every batch - lets attention start ~4x earlier
            for n_ck, ck in enumerate(
                    [b * 4 + ssub for ssub in range(4) for b in range(4)]):
                b = ck // 4
                pair, row = b // 2, (b % 2) * 64
                # one contiguous DMA for the whole chunk (4KB/partition)
                xbf = xp.tile([128, 4 * 512], BF16, tag="xb", name="xbf")
                nc.sync.dma_start(xbf[:], xt_d[:, ck * 2048:(ck + 1) * 2048])
                if n_ck == 0:
                    load_late_weights()
                # Q^T | K^T (stacked 64+64) for this token chunk
                pqk = psA.tile([128, 512], F32, tag="pqk", name="pqk")
                for i in range(4):
                    nc.tensor.matmul(pqk[:], wqk[i][:],
                                     xbf[:, i * 512:(i + 1) * 512],
                                     start=(i == 0), stop=(i == 3))
                scol = (ck % 4) * 512
                nc.scalar.activation(qt[pair][row:row + 64, scol:scol + 512],
                                     pqk[0:64, :], AF.Identity, bias=bqk[0:64, :])
                nc.scalar.activation(kt[pair][row:row + 64, scol:scol + 512],
                                     pqk[64:128, :], AF.Identity, bias=bqk[64:128, :])
                # V (natural layout) per 128-token subchunk; bV is folded
                # into the output-projection bias on the host, so no bias
                # matmul here.
                for sub in range(4):
                    pv = psA.tile([128, DV], F32, tag="pv", name="pv")
                    for i in range(4):
                        nc.tensor.matmul(
                            pv[:], xbf[:, i * 512 + sub * 128:i * 512 + (sub + 1) * 128],
                            wv[i][:], start=(i == 0), stop=(i == 3))
                    tci = ck * 4 + sub  # global token-chunk index (b-major)
                    nc.vector.tensor_copy(vt[:, tci * 65:tci * 65 + 64], pv[:])

        # ================= Phase B: attention (+ phase C half-0 prep) ====
        with tc.tile_pool(name="wc", bufs=2) as wcp:
            # phase-C tiles that must live from mid-attention into the tail
            zc = [wcp.tile([65, N_CORES * HT], BF16, tag=f"zc{q}", name=f"zc{q}")
                  for q in range(2)]
            rd8 = [wcp.tile([8, HT], BF16, tag=f"rd8{q}", name=f"rd8{q}")
                   for q in range(2)]
            rdf = [wcp.tile([8, HT], F32, tag=f"rdf{q}", name=f"rdf{q}")
                   for q in range(2)]
            rfp = [wcp.tile([8, HT], F32, tag=f"rfp{q}", name=f"rfp{q}")
                   for q in range(2)]
            r8b = [wcp.tile([8, HT], BF16, tag=f"r8b{q}", name=f"r8b{q}")
                   for q in range(2)]

            a2a_in_h = [dram.tile([N_CORES * 65, 512], BF16, tag=f"a2a_in{q}",
                                  name=f"a2a_in{q}") for q in range(2)]
            a2a_out_h = [dram.tile([N_CORES * 65, 512], BF16, tag=f"a2a_out{q}",
                                   name=f"a2a_out{q}") for q in range(2)]

            def emit_a2a(q):
                # chunk r = my head's Z^T cols for core r's half-q tokens:
                # tok = (r//2)*S + q*1024 + (r%2)*512 ... +512
                for r in range(N_CORES):
                    col = (r // 2) * S + q * 1024 + (r % 2) * 512
                    nc.sync.dma_start(a2a_in_h[q][r * 65:(r + 1) * 65, :],
                                        zt[:, col:col + 512])
                nc.gpsimd.collective_compute(
                    "AllToAll",
                    mybir.AluOpType.bypass,
                    replica_groups=[list(range(N_CORES))],
                    ins=[a2a_in_h[q][:].opt()],
                    outs=[a2a_out_h[q][:].opt()],
                )

            def gather_half(q):
                """DMA half q out of a2a_out (waits on the collective)."""
                src = a2a_out_h[q][:].rearrange("(j p) s -> p j s", p=65)
                nc.sync.dma_start(rd8[q][:], src[64, :, :])
                nc.sync.dma_start(
                    zc[q][:].rearrange("p (j s) -> p j s", j=N_CORES),
                    src[0:65, :, :])

            def recip_half(q):
                """1/denom on DVE (no PSUM use)."""
                nc.vector.tensor_copy(rdf[q][:], rd8[q][:])
                nc.vector.reciprocal_approx_fast(rfp[q][:], rdf[q][:])
                nc.vector.tensor_copy(r8b[q][:], rfp[q][:])

            with (
                tc.tile_pool(name="wb", bufs=2) as wb,
                tc.tile_pool(name="psB", bufs=1, space="PSUM") as psB,
            ):
                # Software-pipelined over 64 global blocks g = sc*16 + t.
                # Per iteration: scores(g)+exp1(g) are emitted BEFORE the
                # DVE chain of g and exp2(g-1), so the ACT queue interleaves
                # exp1(g+1) ahead of exp2(g) and blocks overlap.
                NB = N_SC * N_TC
                pipe = {}  # g -> p1 tile

                def softmax_b(g):
                    """scores(g) -> e(g) -> p1(g) tiles (no exp2 yet)."""
                    sc, t = g // N_TC, g % N_TC
                    scp = psB.tile([128, 4 * SC], F32, tag="scp", name="scp")
                    for b in range(4):
                        pair, row = b // 2, (b % 2) * 64
                        nc.tensor.matmul(
                            scp[:, b * SC:(b + 1) * SC],
                            kt[pair][row:row + 64, t * TC:(t + 1) * TC],
                            qt[pair][row:row + 64, sc * SC:(sc + 1) * SC],
                            start=True, stop=True,
                        )
                    # e = exp(scores/8) for all 4 b
                    e = wb.tile([128, 4 * SC], BF16, tag="e", name="e", bufs=3)
                    nc.scalar.activation(e[:], scp[:], AF.Exp, scale=0.125)
                    # D = sum_b e ; r = 1/D (custom-DVE fast reciprocal keeps
                    # ACT on the single exp table set - no table thrashing)
                    t01 = wb.tile([128, 2 * SC], BF16, tag="t01", name="t01", bufs=2)
                    nc.vector.tensor_add(t01[:], e[:, 0:2 * SC], e[:, 2 * SC:4 * SC])
                    dd = wb.tile([128, SC], BF16, tag="dd", name="dd", bufs=2)
                    nc.vector.tensor_add(dd[:], t01[:, 0:SC], t01[:, SC:2 * SC])
                    ddf = wb.tile([128, SC], F32, tag="ddf", name="ddf", bufs=2)
                    nc.vector.tensor_copy(ddf[:], dd[:])
                    rf = wb.tile([128, SC], F32, tag="rf", name="rf", bufs=2)
                    nc.vector.reciprocal_approx_fast(rf[:], ddf[:])
                    rr = wb.tile([128, SC], BF16, tag="rr", name="rr", bufs=2)
                    nc.vector.tensor_copy(rr[:], rf[:])
                    # p1 = e * r, one TT with r broadcast along the 4-b free dim
                    p1 = wb.tile([128, 4 * SC], BF16, tag="p1", name="p1", bufs=3)
                    nc.vector.tensor_mul(
                        p1[:].rearrange("p (b s) -> p b s", b=4),
                        e[:].rearrange("p (b s) -> p b s", b=4),
                        rr[:, None, :].broadcast_to([128, 4, SC]),
                    )
                    pipe[g] = p1

                def exp2_and_z(g, zacc):
                    """exp2(g) + Z accumulation (ones-col -> sum_t in row 64)."""
                    t = g % N_TC
                    p1 = pipe.pop(g)
                    q = wb.tile([128, 4 * SC], BF16, tag="q", name="q", bufs=3)
                    nc.scalar.activation(q[:], p1[:], AF.Exp)
                    for b in range(4):
                        tci = b * 16 + t
                        nc.tensor.matmul(
                            zacc[:, b * SC:(b + 1) * SC],
                            vt[:, tci * 65:(tci + 1) * 65],
                            q[:, b * SC:(b + 1) * SC],
                            start=(t == 0), stop=(t == N_TC - 1),
                        )

                zaccs = {}
                for g in range(NB + 2):
                    if g < NB:
                        if g % N_TC == 0:
                            zaccs[g // N_TC] = psB.tile([65, 4 * SC], F32,
                                                        tag="zacc", name="zacc")
                        softmax_b(g)
                    if g >= 2:
                        gz = g - 2
                        za = zaccs[gz // N_TC]
                        exp2_and_z(gz, za)
                        if gz % N_TC == N_TC - 1:
                            # evacuate Z^T (+denominator row) to bf16
                            sc_done = gz // N_TC
                            for b in range(4):
                                col = b * S + sc_done * SC
                                nc.vector.tensor_copy(zt[:, col:col + SC],
                                                      za[:, b * SC:(b + 1) * SC])
                            if sc_done == 1:
                                emit_a2a(0)  # overlaps remaining attention
                            elif sc_done == 3:
                                emit_a2a(1)
                                # a2a_0 done long ago; these run as the
                                # last attention blocks drain
                                gather_half(0)
                                recip_half(0)

            # ============ Phase C: output projection (tail) ============
            with (
                tc.tile_pool(name="oc", bufs=2) as oc,
                tc.tile_pool(name="psC", bufs=2, space="PSUM") as psC,
            ):
                def tail_half(q):
                    # normalized Zc^T in hd-major pairs: tile i = heads 2i,2i+1;
                    # per-head 1/denom broadcast to 64 rows via selector matmul
                    zcn = [wcp.tile([128, HT], BF16, tag=f"zcn{i}",
                                    name=f"zcn{i}", bufs=2) for i in range(4)]
                    for j in range(N_CORES):
                        rbp = psC.tile([64, HT], F32, tag="rbp", name="rbp")
                        nc.tensor.matmul(rbp[:], sel[:, j * 64:(j + 1) * 64],
                                         r8b[q][:], start=True, stop=True)
                        nc.vector.tensor_mul(
                            zcn[j // 2][(j % 2) * 64:(j % 2) * 64 + 64, :],
                            zc[q][0:64, j * HT:(j + 1) * HT],
                            rbp[:],
                        )
                    for m in range(HT // 128):
                        po = psC.tile([128, D], F32, tag="po", name="po")
                        for i in range(4):
                            nc.tensor.matmul(po[:], zcn[i][:, m * 128:(m + 1) * 128],
                                             wo[i][:], start=(i == 0), stop=False)
                        nc.tensor.matmul(po[:], ones_bf[:], bo[:], start=False,
                                         stop=True)
                        ot = oc.tile([128, D], F32, tag="ot", name="ot")
                        nc.scalar.activation(ot[:], po[:], AF.Identity)
                        row = q * HT + m * 128
                        nc.sync.dma_start(out_d[row:row + 128, :], ot[:])

                # half-1's gather waits on a2a_1 inside the DMA queue while
                # half-0 computes; its DVE recip lands after half-0's muls.
                gather_half(1)
                tail_half(0)
                recip_half(1)
                tail_half(1)

    nc.compile()
    return nc


_NC_CACHE = None


def _get_nc():
    global _NC_CACHE
    if _NC_CACHE is None:
        _NC_CACHE = build_kernel()
    return _NC_CACHE


def kernel(X, WQ, bQ, WK, bK, WV, bV, WO, bO, _trace=False, _trace_kwargs=None):
    """Full inputs in, full output out. Shards internally across 8 cores."""
    BF = ml_dtypes.bfloat16
    X = np.asarray(X, dtype=np.float32)
    WQ, bQ = np.asarray(WQ, np.float32), np.asarray(bQ, np.float32)
    WK, bK = np.asarray(WK, np.float32), np.asarray(bK, np.float32)
    WV, bV = np.asarray(WV, np.float32), np.asarray(bV, np.float32)
    WO, bO = np.asarray(WO, np.float32), np.asarray(bO, np.float32)
    # [S,B,D] -> XT [D, NTOK] with b-major tokens (tok = b*S + s), bf16,
    # then pre-tiled [128, chunk(16), dslice(4), 512] so each chunk is one
    # contiguous DMA
    xt = X.transpose(2, 1, 0).reshape(D, NTOK).astype(BF)
    xt = np.ascontiguousarray(
        xt.reshape(4, 128, 16, 512).transpose(1, 2, 0, 3).reshape(128, 16 * 4 * 512))
    wo_b = np.ascontiguousarray(WO.astype(BF))
    # fold the V bias through the output projection:
    # out = (Z/denom) @ WO + (concat_h bV) @ WO + bO
    bo_eff = bV.reshape(H * DV) @ WO + bO
    bo_b = np.ascontiguousarray(bo_eff[None, :].astype(BF))
    sel_np = np.zeros((8, 512), dtype=np.float32)
    for j in range(8):
        sel_np[j, j * 64:(j + 1) * 64] = 1.0
    sel_b = np.ascontiguousarray(sel_np.astype(BF))
    in_maps = []
    for h in range(N_CORES):
        wqk = np.ascontiguousarray(
            np.concatenate([WQ[h], WK[h]], axis=1).astype(BF))
        bqk = np.ascontiguousarray(
            np.concatenate([bQ[h], bK[h]])[:, None], dtype=np.float32)
        in_maps.append({
            "xt": xt,
            "wqk": wqk,
            "bqk": bqk,
            "wv": np.ascontiguousarray(WV[h].astype(BF)),
            "wo": wo_b,
            "bo": bo_b,
            "sel": sel_b,
        })
    nc = _get_nc()
    res = run_bass_kernel_spmd(
        nc, in_maps, core_ids=list(range(N_CORES)),
        trace=_trace, **(_trace_kwargs or {}),
    )
    # core c rows: [0:512] = tokens (c//2)*S + (c%2)*512 .. ; [512:1024] same + 1024
    fullb = np.empty((B, S, D), dtype=np.float32)
    for c in range(N_CORES):
        oc = res.results[c]["out"]
        b, off = c // 2, (c % 2) * 512
        fullb[b, off:off + 512] = oc[0:512]
        fullb[b, 1024 + off:1024 + off + 512] = oc[512:1024]
    full = fullb.transpose(1, 0, 2)
    if _trace:
        return np.ascontiguousarray(full), res
    return np.ascontiguousarray(full)
